# revision 21
# baseline (speedup 1.0000x reference)
"""Trainium2 Bass/Tile SPMD kernel for a 3-layer multimodal LightGCN-style
GNN (segment-sum SpMM message passing + BPR batch lookups).

Strategy (8 NeuronCores):
  - Rows (nodes) sharded uniformly: core c owns rows [c*12500, (c+1)*12500).
    Edges assigned to the owning (destination) core; each core computes its
    own output rows exactly.
  - The three feature tables (E0, image_weight.T, text_weight.T) are packed
    into one [N, 256] bf16 table (192 real cols + 64 pad so gathered rows
    are 512B, a DMA-gather granularity requirement).  bf16 gives 4x PE
    throughput and 1.5x less gather traffic vs f32; PSUM accumulation and
    the running sum S stay f32.
  - Per 128-row block, edges are processed in 128-edge tiles:
      one-hot(localrow)*val built on DVE in bf16 -> PE matmul accumulates
      into PSUM f32.  Gathers are batched per (4-block group, segment):
      ~100 dma_gather ops per layer (vs 784 at per-(block,seg) granularity)
      round-robined over 4 SWDGE queues.
  - After layers 1,2 an AllGather replicates the new bf16 table.
  - Final phase: per-row stats with a single batched Rsqrt (ACT table loads
    once), then owner-computes batch lookups: each core gathers the F rows
    it owns from LOCAL DRAM (no final AllGather); host reassembles.
"""
import os
import sys

import numpy as np

for _p in ("/opt/trn_rl_repo", "/root/.axon_site/_ro/trn_rl_repo"):
    if os.path.isdir(_p) and _p not in sys.path:
        sys.path.append(_p)

import concourse.bass as bass
import concourse.bacc as bacc
import concourse.mybir as mybir
import concourse.tile as tile
from concourse.bass_utils import run_bass_kernel_spmd

P = 128


class Cfg:
    def __init__(self, n_users=50000, n_items=50000, embed=64, n_layers=3,
                 batch=4096, n_cores=8, cat_rate=0.02, seg_rows=25000,
                 group=1):
        self.n_users = n_users
        self.n_items = n_items
        self.N = n_users + n_items
        self.embed = embed
        self.D = 3 * embed                      # 192 real feature cols
        self.DP = 256                           # padded table cols (512B bf16)
        self.n_layers = n_layers
        self.batch = batch
        self.NC = n_cores
        self.cat_rate = cat_rate
        assert self.N % n_cores == 0
        self.RPC = self.N // n_cores            # rows per core
        self.NB = (self.RPC + P - 1) // P       # row blocks per core
        assert seg_rows <= 32767
        self.SEG = seg_rows                     # table rows per gather segment
        self.NSEG = -(-self.N // seg_rows)
        self.G = group                          # blocks per gather group
        self.NGRP = -(-self.NB // group)


def preprocess(cfg, inputs):
    """Host layout prep. Returns (meta, in_maps)."""
    N, D, DP, E64 = cfg.N, cfg.D, cfg.DP, cfg.embed
    NC, RPC, NB, SEG, NSEG, G = cfg.NC, cfg.RPC, cfg.NB, cfg.SEG, cfg.NSEG, \
        cfg.G

    rows = np.asarray(inputs["adj_rows"]).astype(np.int64)
    cols = np.asarray(inputs["adj_cols"]).astype(np.int64)
    vals = np.asarray(inputs["adj_vals"]).astype(np.float32)
    E0 = np.asarray(inputs["E0"]).astype(np.float32)
    iw = np.asarray(inputs["image_weight"]).astype(np.float32)
    ib = np.asarray(inputs["image_bias"]).astype(np.float32)
    tw = np.asarray(inputs["text_weight"]).astype(np.float32)
    tb = np.asarray(inputs["text_bias"]).astype(np.float32)
    uidx = np.asarray(inputs["user_indices"]).astype(np.int64)
    pidx = np.asarray(inputs["pos_item_indices"]).astype(np.int64)
    nidx = np.asarray(inputs["neg_item_indices"]).astype(np.int64)

    # packed bf16 table [N, 256]: [E0 | img.T | txt.T | 0pad]
    import ml_dtypes
    X0 = np.zeros((N, DP), dtype=ml_dtypes.bfloat16)
    X0[:, 0:E64] = E0
    X0[:, E64:2 * E64] = iw.T
    X0[:, 2 * E64:3 * E64] = tw.T
    bias192 = np.concatenate([np.zeros(E64, np.float32), ib, tb])
    bias_full = np.broadcast_to(bias192[None, :], (P, D)).copy()
    iota = np.broadcast_to(np.arange(P, dtype=np.float32)[None, :],
                           (P, P)).copy()

    core = rows // RPC
    lrow = rows % RPC                     # local row within core
    blk = lrow // P                       # block within core
    grp = blk // G
    seg = cols // SEG
    order = np.lexsort((cols, blk, seg, grp, core))
    core_s = core[order]
    blk_s = blk[order]
    seg_s = seg[order]
    cols_s = cols[order]
    vals_s = vals[order]
    lrow_s = (lrow % P)[order]            # row within block

    # per (core, block, seg) counts -> shared padded sizes
    counts = np.zeros((NC, NB, NSEG), np.int64)
    np.add.at(counts, (core, blk, seg), 1)
    npad = -(-counts.max(axis=0) // P) * P           # [NB, NSEG]
    # ensure every block has >=1 tile so PSUM gets initialized
    for b in range(NB):
        if npad[b].sum() == 0:
            npad[b, 0] = P

    # slot layout: for g, for s, for b in g : npad[b, s] idx slots.
    # dma_gather ops are capped at MAXI idx (SWDGE descriptor-ring limit:
    # >1024 descriptors per op hangs the DGE).
    MAXI = 1024
    start_idx = np.zeros((NB, NSEG), np.int64)
    gather_ops = []        # per group: list of (start_slot_idx, n_idx, seg)
    acc = 0
    for g in range(cfg.NGRP):
        bs = range(g * G, min((g + 1) * G, NB))
        grp_ops = []
        for s in range(NSEG):
            op_start = acc
            for b in bs:
                start_idx[b, s] = acc
                acc += int(npad[b, s])
            n = acc - op_start
            o = op_start
            while n > 0:
                c = min(n, MAXI)
                grp_ops.append((o, c, s))
                o += c
                n -= c
        gather_ops.append(grp_ops)
    NI = acc                                     # total idx slots per core
    T = NI // P                                  # total 128-edge tiles
    # per-block tile ranges [(tile0, ntiles), ...] in seg order
    block_tiles = []
    for b in range(NB):
        tr = [(int(start_idx[b, s]) // P, int(npad[b, s]) // P)
              for s in range(NSEG) if npad[b, s] > 0]
        block_tiles.append(tr)

    # ---- batch lookups: owner-computes ----
    gl = np.concatenate([uidx, cfg.n_users + pidx, cfg.n_users + nidx])
    owner = gl // RPC
    local = (gl % RPC).astype(np.int64)
    bcounts = np.bincount(owner, minlength=NC)
    NH = int(-(-bcounts.max() // P) * P)
    border = np.argsort(owner, kind="stable")

    meta = dict(npad=npad, gather_ops=gather_ops, block_tiles=block_tiles,
                T=T, NI=NI, NH=NH)

    # ---- per-core input slabs ----
    XCOLS = NI // 16
    # per-edge destination slot: start_idx[blk,seg] + rank within (c,b,s) run
    run_key = (core_s * NB + blk_s) * NSEG + seg_s
    run_change = np.empty(len(run_key), np.bool_)
    run_change[0] = True
    run_change[1:] = run_key[1:] != run_key[:-1]
    run_first = np.flatnonzero(run_change)
    run_id = np.cumsum(run_change) - 1
    rank = np.arange(len(run_key)) - run_first[run_id]
    slot = start_idx[blk_s, seg_s] + rank

    in_maps = []
    hb = []          # host bookkeeping for assemble
    for c in range(NC):
        m = (core_s == c)
        idx16 = np.zeros((NI,), np.int16)
        evals_f = np.zeros((NI,), np.float32)
        lrow_f = np.zeros((NI,), np.float32)
        sl = slot[m]
        idx16[sl] = (cols_s[m] - seg_s[m] * SEG).astype(np.int16)
        evals_f[sl] = vals_s[m]
        lrow_f[sl] = lrow_s[m]
        idx_full = np.tile(idx16.reshape(XCOLS, 16).T, (8, 1))
        evals_sl = evals_f.reshape(T, P).T.copy()
        lrow_sl = lrow_f.reshape(T, P).T.copy()

        x0c = np.zeros((NB * P, D), np.float32)
        x0c[:RPC, 0:E64] = E0[c * RPC:(c + 1) * RPC]
        x0c[:RPC, E64:2 * E64] = iw.T[c * RPC:(c + 1) * RPC]
        x0c[:RPC, 2 * E64:3 * E64] = tw.T[c * RPC:(c + 1) * RPC]
        x0loc = np.ascontiguousarray(
            x0c.reshape(NB, P, D).transpose(1, 0, 2).reshape(P, NB * D))

        # owned batch indices (padded to NH)
        sel = border[owner[border] == c]
        bl16 = np.zeros((NH,), np.int16)
        bl16[:len(sel)] = local[sel].astype(np.int16)
        bidx_full = np.tile(bl16.reshape(NH // 16, 16).T, (8, 1))
        hb.append(sel)

        in_maps.append({
            "gidx": np.ascontiguousarray(idx_full),
            "evals": evals_sl,
            "lrow": lrow_sl,
            "biasf": bias_full,
            "identf": np.eye(P, dtype=ml_dtypes.bfloat16),
            "biasb": bias_full.astype(ml_dtypes.bfloat16),
            "iota": iota,
            "x0loc": x0loc,
            "bidx": np.ascontiguousarray(bidx_full),
            "xt": X0,
        })
    meta["hb"] = hb
    return meta, in_maps


def build_program(cfg, meta, skip_ag=False, skip_pe=False, skip_gather=False):
    """Build the SPMD Bass program shared by all cores.

    skip_* flags are for ablation timing only (results wrong)."""
    N, D, DP = cfg.N, cfg.D, cfg.DP
    NC, RPC, NB, SEG, NSEG = cfg.NC, cfg.RPC, cfg.NB, cfg.SEG, cfg.NSEG
    NL = cfg.n_layers
    T, NI, NH = meta["T"], meta["NI"], meta["NH"]
    gather_ops = meta["gather_ops"]
    block_tiles = meta["block_tiles"]
    XCOLS = NI // 16
    NHT = NH // P
    f32 = mybir.dt.float32
    bf16 = mybir.dt.bfloat16

    nc = bacc.Bacc("TRN2", num_devices=NC, debug=False, num_swdge_queues=4)
    xt = nc.dram_tensor("xt", [N, DP], bf16, kind="ExternalInput")
    gidx = nc.dram_tensor("gidx", [P, XCOLS], mybir.dt.int16,
                          kind="ExternalInput")
    evals = nc.dram_tensor("evals", [P, T], f32, kind="ExternalInput")
    lrow = nc.dram_tensor("lrow", [P, T], f32, kind="ExternalInput")
    biasf = nc.dram_tensor("biasf", [P, D], f32, kind="ExternalInput")
    identf = nc.dram_tensor("identf", [P, P], bf16, kind="ExternalInput")
    biasb = nc.dram_tensor("biasb", [P, D], bf16, kind="ExternalInput")
    iota = nc.dram_tensor("iota", [P, P], f32, kind="ExternalInput")
    x0loc = nc.dram_tensor("x0loc", [P, NB * D], f32, kind="ExternalInput")
    bidx = nc.dram_tensor("bidx", [P, NH // 16], mybir.dt.int16,
                          kind="ExternalInput")
    bout = nc.dram_tensor("bout", [NH, D], f32, kind="ExternalOutput")

    rg = [list(range(NC))]
    inv = 1.0 / (NL + 1)
    E64 = cfg.embed
    sc = 1.0 / (cfg.cat_rate * cfg.cat_rate)

    with tile.TileContext(nc) as tc:
        with tc.tile_pool(name="const", bufs=1) as cpool, \
             tc.tile_pool(name="g", bufs=14) as gpool, \
             tc.tile_pool(name="h", bufs=10) as hpool, \
             tc.tile_pool(name="e", bufs=6) as epool, \
             tc.tile_pool(name="f", bufs=4) as fpool, \
             tc.tile_pool(name="ps", bufs=8, space="PSUM") as pspool, \
             tc.tile_pool(name="dram", bufs=1, space="DRAM") as dram:
            gidx_sb = cpool.tile([P, XCOLS], mybir.dt.int16)
            nc.sync.dma_start(out=gidx_sb[:], in_=gidx[:])
            evals_sb = cpool.tile([P, T], f32)
            nc.sync.dma_start(out=evals_sb[:], in_=evals[:])
            lrow_sb = cpool.tile([P, T], f32)
            nc.sync.dma_start(out=lrow_sb[:], in_=lrow[:])
            bias_sb = cpool.tile([P, D], f32)
            nc.sync.dma_start(out=bias_sb[:], in_=biasf[:])
            ident_sb = cpool.tile([P, P], bf16)
            nc.sync.dma_start(out=ident_sb[:], in_=identf[:])
            biasb_sb = cpool.tile([P, D], bf16)
            nc.sync.dma_start(out=biasb_sb[:], in_=biasb[:])
            iota_sb = cpool.tile([P, P], f32)
            nc.sync.dma_start(out=iota_sb[:], in_=iota[:])
            S_sb = cpool.tile([P, NB * D], f32)
            nc.sync.dma_start(out=S_sb[:], in_=x0loc[:])
            bidx_sb = cpool.tile([P, NH // 16], mybir.dt.int16)
            nc.sync.dma_start(out=bidx_sb[:], in_=bidx[:])
            rr_all = cpool.tile([P, 2 * NB], f32)
            rs_all = cpool.tile([P, 2 * NB], f32)

            ag_in = dram.tile([RPC, DP], bf16)
            Xa = dram.tile([N, DP], bf16, addr_space="Shared")
            Xb = dram.tile([N, DP], bf16, addr_space="Shared")
            Fin = dram.tile([RPC, D], f32)

            sources = [xt, xt, xt] if skip_ag else [xt, Xa, Xb]

            def close_block(b, ps, layer):
                """Post-matmul per-block work, issued one block late so the
                in-order DVE queue never stalls waiting on PE."""
                rows_b = min(RPC - b * P, P)
                Ssl = S_sb[:, b * D:(b + 1) * D]
                nc.vector.tensor_tensor(out=Ssl, in0=Ssl, in1=ps[:],
                                        op=mybir.AluOpType.add)
                if layer < NL - 1:
                    xb = epool.tile([P, DP], bf16, tag="xb")
                    nc.scalar.activation(
                        out=xb[:, 0:D], in_=ps[:],
                        func=mybir.ActivationFunctionType.Copy)
                    nc.sync.dma_start(
                        out=ag_in[b * P:b * P + rows_b, :],
                        in_=xb[:rows_b, :])
                else:
                    # norm stats (pass 1 of final phase)
                    tmp = fpool.tile([P, 2 * E64], f32, tag="tmp")
                    nc.vector.tensor_tensor(
                        out=tmp[:], in0=Ssl[:, E64:3 * E64],
                        in1=Ssl[:, E64:3 * E64],
                        op=mybir.AluOpType.mult)
                    nc.vector.reduce_sum(
                        out=rr_all[:, 2 * b:2 * b + 1],
                        in_=tmp[:, 0:E64], axis=mybir.AxisListType.X)
                    nc.vector.reduce_sum(
                        out=rr_all[:, 2 * b + 1:2 * b + 2],
                        in_=tmp[:, E64:2 * E64], axis=mybir.AxisListType.X)

            for layer in range(NL):
                src = sources[layer]
                qn = 0
                pend = None
                tile_src = [None] * T
                for g in range(cfg.NGRP):
                    for (o_start, o_n, s) in gather_ops[g]:
                        nsl = o_n // P
                        gt = gpool.tile([P, nsl * DP], bf16, tag="g")
                        if skip_gather:
                            nc.vector.memset(gt[:, 0:1], 0.0)
                        else:
                            nc.gpsimd.dma_gather(
                                out_ap=gt[:].rearrange("p (t e) -> p t e",
                                                       t=nsl),
                                in_ap=src[s * SEG:min((s + 1) * SEG, N), :],
                                idxs_ap=gidx_sb[:, o_start // 16:
                                                (o_start + o_n) // 16],
                                num_idxs=o_n,
                                num_idxs_reg=o_n,
                                elem_size=DP,
                                queue_num=qn,
                            )
                        qn = (qn + 1) % 4
                        for k in range(nsl):
                            tile_src[o_start // P + k] = (gt, k)
                    for b in range(g * cfg.G, min((g + 1) * cfg.G, NB)):
                            tl = []
                            for (t0, nt) in block_tiles[b]:
                                tl.extend(range(t0, t0 + nt))
                            ps = pspool.tile([P, D], f32, space="PSUM",
                                             tag="ps")
                            tl_eff = tl[:1] if skip_pe else tl
                            for j, t in enumerate(tl_eff):
                                sh = hpool.tile([P, P], bf16, tag="sh")
                                nc.vector.tensor_scalar(
                                    out=sh[:], in0=iota_sb[:],
                                    scalar1=lrow_sb[:, t:t + 1],
                                    scalar2=evals_sb[:, t:t + 1],
                                    op0=mybir.AluOpType.is_equal,
                                    op1=mybir.AluOpType.mult)
                                gt, kg = tile_src[t]
                                nc.tensor.matmul(
                                    out=ps[:], lhsT=sh[:],
                                    rhs=gt[:, kg * DP:kg * DP + D],
                                    start=(j == 0), stop=False)
                            # bias add on PE: identity lhsT x broadcast bias
                            nc.tensor.matmul(
                                out=ps[:], lhsT=ident_sb[:], rhs=biasb_sb[:],
                                start=False, stop=True)
                            # close of the PREVIOUS block: keeps the in-order
                            # DVE queue from stalling on this block's matmuls
                            if pend is not None:
                                close_block(pend[0], pend[1], layer)
                            pend = (b, ps)
                if pend is not None:
                    close_block(pend[0], pend[1], layer)
                    pend = None
                if layer < NL - 1 and not skip_ag:
                    dst = sources[layer + 1]
                    nc.gpsimd.collective_compute(
                        "AllGather", mybir.AluOpType.bypass,
                        replica_groups=rg, ins=[ag_in[:]], outs=[dst[:]])

            # ---- final phase pass 2 ----
            nc.scalar.activation(out=rs_all[:], in_=rr_all[:],
                                 func=mybir.ActivationFunctionType.Sqrt,
                                 scale=sc)
            nc.vector.reciprocal(out=rs_all[:], in_=rs_all[:])
            for b in range(NB):
                rows_b = min(RPC - b * P, P)
                Sb = S_sb[:, b * D:(b + 1) * D]
                F_sb = fpool.tile([P, D], f32, tag="F")
                t2 = fpool.tile([P, 2 * E64], f32, tag="t2")
                nc.vector.tensor_scalar(
                    out=t2[:, 0:E64], in0=Sb[:, E64:2 * E64],
                    scalar1=rs_all[:, 2 * b:2 * b + 1], scalar2=None,
                    op0=mybir.AluOpType.mult)
                nc.vector.tensor_scalar(
                    out=t2[:, E64:2 * E64], in0=Sb[:, 2 * E64:3 * E64],
                    scalar1=rs_all[:, 2 * b + 1:2 * b + 2], scalar2=None,
                    op0=mybir.AluOpType.mult)
                nc.vector.tensor_scalar(
                    out=F_sb[:], in0=Sb[:], scalar1=inv, scalar2=None,
                    op0=mybir.AluOpType.mult)
                nc.vector.tensor_tensor(
                    out=t2[:, 0:E64], in0=t2[:, 0:E64],
                    in1=t2[:, E64:2 * E64], op=mybir.AluOpType.add)
                nc.vector.tensor_tensor(
                    out=F_sb[:, 0:E64], in0=F_sb[:, 0:E64],
                    in1=t2[:, 0:E64], op=mybir.AluOpType.add)
                nc.sync.dma_start(out=Fin[b * P:b * P + rows_b, :],
                                  in_=F_sb[:rows_b, :])

            # ---- owner-computes batch gathers from local Fin ----
            fgt = cpool.tile([P, NHT * D], f32)
            qn = 0
            o = 0
            while o < NH:
                c = min(NH - o, 1024)
                nc.gpsimd.dma_gather(
                    out_ap=fgt[:, o // P * D:(o + c) // P * D].rearrange(
                        "p (t e) -> p t e", t=c // P),
                    in_ap=Fin[:],
                    idxs_ap=bidx_sb[:, o // 16:(o + c) // 16],
                    num_idxs=c,
                    num_idxs_reg=c,
                    elem_size=D,
                    queue_num=qn,
                )
                qn = (qn + 1) % 4
                o += c
            for t in range(NHT):
                nc.sync.dma_start(
                    out=bout[t * P:(t + 1) * P, :],
                    in_=fgt[:, t * D:(t + 1) * D])
    nc.compile()
    return nc


_CACHE = {}


def _get_program(cfg, meta):
    key = (meta["npad"].tobytes(), meta["NH"], cfg.N, cfg.D, cfg.NC)
    if key not in _CACHE:
        _CACHE[key] = build_program(cfg, meta)
    return _CACHE[key]


def run(cfg, inputs):
    meta, in_maps = preprocess(cfg, inputs)
    nc = _get_program(cfg, meta)
    res = run_bass_kernel_spmd(nc, in_maps, core_ids=list(range(cfg.NC)))
    return assemble(cfg, meta, res.results)


def assemble(cfg, meta, results):
    D, E64 = cfg.D, cfg.embed
    B = cfg.batch
    full = np.zeros((3 * B, D), np.float32)
    for c in range(cfg.NC):
        sel = meta["hb"][c]
        full[sel] = results[c]["bout"][:len(sel)]
    out = []
    for part in range(3):          # combined, mean_img, mean_txt
        for s in range(3):         # user, pos, neg
            out.append(np.ascontiguousarray(
                full[s * B:(s + 1) * B, part * E64:(part + 1) * E64]))
    return tuple(out)


def kernel(**inputs):
    cfg = Cfg()
    return run(cfg, inputs)


# revision 22
# speedup vs baseline: 1.0422x; 1.0422x over previous
"""Trainium2 Bass/Tile SPMD kernel for a 3-layer multimodal LightGCN-style
GNN (segment-sum SpMM message passing + BPR batch lookups).

Strategy (8 NeuronCores):
  - Rows (nodes) sharded uniformly: core c owns rows [c*12500, (c+1)*12500).
    Edges assigned to the owning (destination) core; each core computes its
    own output rows exactly.
  - The three feature tables (E0, image_weight.T, text_weight.T) are packed
    into one [N, 256] bf16 table (192 real cols + 64 pad so gathered rows
    are 512B, a DMA-gather granularity requirement).  bf16 gives 4x PE
    throughput and 1.5x less gather traffic vs f32; PSUM accumulation and
    the running sum S stay f32.
  - Per 128-row block, edges are processed in 128-edge tiles:
      one-hot(localrow)*val built on DVE in bf16 -> PE matmul accumulates
      into PSUM f32.  Gathers are batched per (4-block group, segment):
      ~100 dma_gather ops per layer (vs 784 at per-(block,seg) granularity)
      round-robined over 4 SWDGE queues.
  - After layers 1,2 an AllGather replicates the new bf16 table.
  - Final phase: per-row stats with a single batched Rsqrt (ACT table loads
    once), then owner-computes batch lookups: each core gathers the F rows
    it owns from LOCAL DRAM (no final AllGather); host reassembles.
"""
import os
import sys

import numpy as np

for _p in ("/opt/trn_rl_repo", "/root/.axon_site/_ro/trn_rl_repo"):
    if os.path.isdir(_p) and _p not in sys.path:
        sys.path.append(_p)

import concourse.bass as bass
import concourse.bacc as bacc
import concourse.mybir as mybir
import concourse.tile as tile
from concourse.bass_utils import run_bass_kernel_spmd

P = 128


class Cfg:
    def __init__(self, n_users=50000, n_items=50000, embed=64, n_layers=3,
                 batch=4096, n_cores=8, cat_rate=0.02, seg_rows=25000,
                 group=1):
        self.n_users = n_users
        self.n_items = n_items
        self.N = n_users + n_items
        self.embed = embed
        self.D = 3 * embed                      # 192 real feature cols
        self.DP = 256                           # padded table cols (512B bf16)
        self.n_layers = n_layers
        self.batch = batch
        self.NC = n_cores
        self.cat_rate = cat_rate
        assert self.N % n_cores == 0
        self.RPC = self.N // n_cores            # rows per core
        self.NB = (self.RPC + P - 1) // P       # row blocks per core
        assert seg_rows <= 32767
        self.SEG = seg_rows                     # table rows per gather segment
        self.NSEG = -(-self.N // seg_rows)
        self.G = group                          # blocks per gather group
        self.NGRP = -(-self.NB // group)


def preprocess(cfg, inputs):
    """Host layout prep. Returns (meta, in_maps)."""
    N, D, DP, E64 = cfg.N, cfg.D, cfg.DP, cfg.embed
    NC, RPC, NB, SEG, NSEG, G = cfg.NC, cfg.RPC, cfg.NB, cfg.SEG, cfg.NSEG, \
        cfg.G

    rows = np.asarray(inputs["adj_rows"]).astype(np.int64)
    cols = np.asarray(inputs["adj_cols"]).astype(np.int64)
    vals = np.asarray(inputs["adj_vals"]).astype(np.float32)
    E0 = np.asarray(inputs["E0"]).astype(np.float32)
    iw = np.asarray(inputs["image_weight"]).astype(np.float32)
    ib = np.asarray(inputs["image_bias"]).astype(np.float32)
    tw = np.asarray(inputs["text_weight"]).astype(np.float32)
    tb = np.asarray(inputs["text_bias"]).astype(np.float32)
    uidx = np.asarray(inputs["user_indices"]).astype(np.int64)
    pidx = np.asarray(inputs["pos_item_indices"]).astype(np.int64)
    nidx = np.asarray(inputs["neg_item_indices"]).astype(np.int64)

    # packed bf16 table [N, 256]: [E0 | img.T | txt.T | 0pad]
    import ml_dtypes
    X0 = np.zeros((N, DP), dtype=ml_dtypes.bfloat16)
    X0[:, 0:E64] = E0
    X0[:, E64:2 * E64] = iw.T
    X0[:, 2 * E64:3 * E64] = tw.T
    bias192 = np.concatenate([np.zeros(E64, np.float32), ib, tb])
    bias_full = np.broadcast_to(bias192[None, :], (P, D)).copy()
    iota = np.broadcast_to(np.arange(P, dtype=np.float32)[None, :],
                           (P, P)).copy()

    core = rows // RPC
    lrow = rows % RPC                     # local row within core
    blk = lrow // P                       # block within core
    grp = blk // G
    seg = cols // SEG
    order = np.lexsort((cols, blk, seg, grp, core))
    core_s = core[order]
    blk_s = blk[order]
    seg_s = seg[order]
    cols_s = cols[order]
    vals_s = vals[order]
    lrow_s = (lrow % P)[order]            # row within block

    # per (core, block, seg) counts -> shared padded sizes
    counts = np.zeros((NC, NB, NSEG), np.int64)
    np.add.at(counts, (core, blk, seg), 1)
    npad = -(-counts.max(axis=0) // P) * P           # [NB, NSEG]
    # ensure every block has >=1 tile so PSUM gets initialized
    for b in range(NB):
        if npad[b].sum() == 0:
            npad[b, 0] = P

    # slot layout: for g, for s, for b in g : npad[b, s] idx slots.
    # dma_gather ops are capped at MAXI idx (SWDGE descriptor-ring limit:
    # >1024 descriptors per op hangs the DGE).
    MAXI = 1024
    start_idx = np.zeros((NB, NSEG), np.int64)
    gather_ops = []        # per group: list of (start_slot_idx, n_idx, seg)
    acc = 0
    for g in range(cfg.NGRP):
        bs = range(g * G, min((g + 1) * G, NB))
        grp_ops = []
        for s in range(NSEG):
            op_start = acc
            for b in bs:
                start_idx[b, s] = acc
                acc += int(npad[b, s])
            n = acc - op_start
            o = op_start
            while n > 0:
                c = min(n, MAXI)
                grp_ops.append((o, c, s))
                o += c
                n -= c
        gather_ops.append(grp_ops)
    NI = acc                                     # total idx slots per core
    T = NI // P                                  # total 128-edge tiles
    # per-block tile ranges [(tile0, ntiles), ...] in seg order
    block_tiles = []
    for b in range(NB):
        tr = [(int(start_idx[b, s]) // P, int(npad[b, s]) // P)
              for s in range(NSEG) if npad[b, s] > 0]
        block_tiles.append(tr)

    # ---- batch lookups: owner-computes ----
    gl = np.concatenate([uidx, cfg.n_users + pidx, cfg.n_users + nidx])
    owner = gl // RPC
    local = (gl % RPC).astype(np.int64)
    bcounts = np.bincount(owner, minlength=NC)
    NH = int(-(-bcounts.max() // P) * P)
    border = np.argsort(owner, kind="stable")

    meta = dict(npad=npad, gather_ops=gather_ops, block_tiles=block_tiles,
                T=T, NI=NI, NH=NH)

    # ---- per-core input slabs ----
    XCOLS = NI // 16
    # per-edge destination slot: start_idx[blk,seg] + rank within (c,b,s) run
    run_key = (core_s * NB + blk_s) * NSEG + seg_s
    run_change = np.empty(len(run_key), np.bool_)
    run_change[0] = True
    run_change[1:] = run_key[1:] != run_key[:-1]
    run_first = np.flatnonzero(run_change)
    run_id = np.cumsum(run_change) - 1
    rank = np.arange(len(run_key)) - run_first[run_id]
    slot = start_idx[blk_s, seg_s] + rank

    in_maps = []
    hb = []          # host bookkeeping for assemble
    for c in range(NC):
        m = (core_s == c)
        idx16 = np.zeros((NI,), np.int16)
        evals_f = np.zeros((NI,), np.float32)
        lrow_f = np.zeros((NI,), np.float32)
        sl = slot[m]
        idx16[sl] = (cols_s[m] - seg_s[m] * SEG).astype(np.int16)
        evals_f[sl] = vals_s[m]
        lrow_f[sl] = lrow_s[m]
        idx_full = np.tile(idx16.reshape(XCOLS, 16).T, (8, 1))
        evals_sl = evals_f.reshape(T, P).T.copy()
        lrow_sl = lrow_f.reshape(T, P).T.copy()

        x0c = np.zeros((NB * P, D), np.float32)
        x0c[:RPC, 0:E64] = E0[c * RPC:(c + 1) * RPC]
        x0c[:RPC, E64:2 * E64] = iw.T[c * RPC:(c + 1) * RPC]
        x0c[:RPC, 2 * E64:3 * E64] = tw.T[c * RPC:(c + 1) * RPC]
        x0loc = np.ascontiguousarray(
            x0c.reshape(NB, P, D).transpose(1, 0, 2).reshape(P, NB * D))

        # owned batch indices (padded to NH)
        sel = border[owner[border] == c]
        bl16 = np.zeros((NH,), np.int16)
        bl16[:len(sel)] = local[sel].astype(np.int16)
        bidx_full = np.tile(bl16.reshape(NH // 16, 16).T, (8, 1))
        hb.append(sel)

        in_maps.append({
            "gidx": np.ascontiguousarray(idx_full),
            "evals": evals_sl,
            "lrow": lrow_sl,
            "biasf": bias_full,
            "identf": np.eye(P, dtype=ml_dtypes.bfloat16),
            "biasb": bias_full.astype(ml_dtypes.bfloat16),
            "iota": iota,
            "x0loc": x0loc,
            "bidx": np.ascontiguousarray(bidx_full),
            "xt": X0,
        })
    meta["hb"] = hb
    return meta, in_maps


def build_program(cfg, meta, skip_ag=False, skip_pe=False, skip_gather=False):
    """Build the SPMD Bass program shared by all cores.

    skip_* flags are for ablation timing only (results wrong)."""
    N, D, DP = cfg.N, cfg.D, cfg.DP
    NC, RPC, NB, SEG, NSEG = cfg.NC, cfg.RPC, cfg.NB, cfg.SEG, cfg.NSEG
    NL = cfg.n_layers
    T, NI, NH = meta["T"], meta["NI"], meta["NH"]
    gather_ops = meta["gather_ops"]
    block_tiles = meta["block_tiles"]
    XCOLS = NI // 16
    NHT = NH // P
    f32 = mybir.dt.float32
    bf16 = mybir.dt.bfloat16

    nc = bacc.Bacc("TRN2", num_devices=NC, debug=False, num_swdge_queues=4)
    xt = nc.dram_tensor("xt", [N, DP], bf16, kind="ExternalInput")
    gidx = nc.dram_tensor("gidx", [P, XCOLS], mybir.dt.int16,
                          kind="ExternalInput")
    evals = nc.dram_tensor("evals", [P, T], f32, kind="ExternalInput")
    lrow = nc.dram_tensor("lrow", [P, T], f32, kind="ExternalInput")
    biasf = nc.dram_tensor("biasf", [P, D], f32, kind="ExternalInput")
    identf = nc.dram_tensor("identf", [P, P], bf16, kind="ExternalInput")
    biasb = nc.dram_tensor("biasb", [P, D], bf16, kind="ExternalInput")
    iota = nc.dram_tensor("iota", [P, P], f32, kind="ExternalInput")
    x0loc = nc.dram_tensor("x0loc", [P, NB * D], f32, kind="ExternalInput")
    bidx = nc.dram_tensor("bidx", [P, NH // 16], mybir.dt.int16,
                          kind="ExternalInput")
    bout = nc.dram_tensor("bout", [NH, D], f32, kind="ExternalOutput")

    rg = [list(range(NC))]
    inv = 1.0 / (NL + 1)
    E64 = cfg.embed
    sc = 1.0 / (cfg.cat_rate * cfg.cat_rate)

    with tile.TileContext(nc) as tc:
        with tc.tile_pool(name="const", bufs=1) as cpool, \
             tc.tile_pool(name="g", bufs=12) as gpool, \
             tc.tile_pool(name="h", bufs=10) as hpool, \
             tc.tile_pool(name="e", bufs=6) as epool, \
             tc.tile_pool(name="f", bufs=4) as fpool, \
             tc.tile_pool(name="ps", bufs=8, space="PSUM") as pspool, \
             tc.tile_pool(name="dram", bufs=1, space="DRAM") as dram:
            gidx_sb = cpool.tile([P, XCOLS], mybir.dt.int16)
            nc.sync.dma_start(out=gidx_sb[:], in_=gidx[:])
            evals_sb = cpool.tile([P, T], f32)
            nc.sync.dma_start(out=evals_sb[:], in_=evals[:])
            lrow_sb = cpool.tile([P, T], f32)
            nc.sync.dma_start(out=lrow_sb[:], in_=lrow[:])
            bias_sb = cpool.tile([P, D], f32)
            nc.sync.dma_start(out=bias_sb[:], in_=biasf[:])
            ident_sb = cpool.tile([P, P], bf16)
            nc.sync.dma_start(out=ident_sb[:], in_=identf[:])
            biasb_sb = cpool.tile([P, D], bf16)
            nc.sync.dma_start(out=biasb_sb[:], in_=biasb[:])
            iota_sb = cpool.tile([P, P], f32)
            nc.sync.dma_start(out=iota_sb[:], in_=iota[:])
            S_sb = cpool.tile([P, NB * D], f32)
            nc.sync.dma_start(out=S_sb[:], in_=x0loc[:])
            bidx_sb = cpool.tile([P, NH // 16], mybir.dt.int16)
            nc.sync.dma_start(out=bidx_sb[:], in_=bidx[:])
            rr_all = cpool.tile([P, 2 * NB], f32)
            rs_all = cpool.tile([P, 2 * NB], f32)

            ag_in = dram.tile([RPC, DP], bf16)
            Xa = dram.tile([N, DP], bf16, addr_space="Shared")
            Xb = dram.tile([N, DP], bf16, addr_space="Shared")
            Fin = dram.tile([RPC, D], f32)

            sources = [xt, xt, xt] if skip_ag else [xt, Xa, Xb]

            def close_block(b, ps, layer):
                """Post-matmul per-block work, issued one block late so the
                in-order DVE queue never stalls waiting on PE."""
                rows_b = min(RPC - b * P, P)
                Ssl = S_sb[:, b * D:(b + 1) * D]
                nc.vector.tensor_tensor(out=Ssl, in0=Ssl, in1=ps[:],
                                        op=mybir.AluOpType.add)
                if layer < NL - 1:
                    xb = epool.tile([P, DP], bf16, tag="xb")
                    nc.scalar.activation(
                        out=xb[:, 0:D], in_=ps[:],
                        func=mybir.ActivationFunctionType.Copy)
                    nc.sync.dma_start(
                        out=ag_in[b * P:b * P + rows_b, :],
                        in_=xb[:rows_b, :])
                else:
                    # norm stats (pass 1 of final phase)
                    tmp = fpool.tile([P, 2 * E64], f32, tag="tmp")
                    nc.vector.tensor_tensor(
                        out=tmp[:], in0=Ssl[:, E64:3 * E64],
                        in1=Ssl[:, E64:3 * E64],
                        op=mybir.AluOpType.mult)
                    nc.vector.reduce_sum(
                        out=rr_all[:, 2 * b:2 * b + 1],
                        in_=tmp[:, 0:E64], axis=mybir.AxisListType.X)
                    nc.vector.reduce_sum(
                        out=rr_all[:, 2 * b + 1:2 * b + 2],
                        in_=tmp[:, E64:2 * E64], axis=mybir.AxisListType.X)

            for layer in range(NL):
                src = sources[layer]
                qn = 0
                pend = None
                tile_src = [None] * T
                for g in range(cfg.NGRP):
                    for (o_start, o_n, s) in gather_ops[g]:
                        nsl = o_n // P
                        gt = gpool.tile([P, nsl * DP], bf16, tag="g")
                        if skip_gather:
                            nc.vector.memset(gt[:, 0:1], 0.0)
                        else:
                            nc.gpsimd.dma_gather(
                                out_ap=gt[:].rearrange("p (t e) -> p t e",
                                                       t=nsl),
                                in_ap=src[s * SEG:min((s + 1) * SEG, N), :],
                                idxs_ap=gidx_sb[:, o_start // 16:
                                                (o_start + o_n) // 16],
                                num_idxs=o_n,
                                num_idxs_reg=o_n,
                                elem_size=DP,
                                queue_num=qn,
                            )
                        qn = (qn + 1) % 4
                        for k in range(nsl):
                            tile_src[o_start // P + k] = (gt, k)
                    for b in range(g * cfg.G, min((g + 1) * cfg.G, NB)):
                            tl = []
                            for (t0, nt) in block_tiles[b]:
                                tl.extend(range(t0, t0 + nt))
                            ps = pspool.tile([P, D], f32, space="PSUM",
                                             tag="ps")
                            tl_eff = tl[:1] if skip_pe else tl
                            for j, t in enumerate(tl_eff):
                                sh = hpool.tile([P, P], bf16, tag="sh")
                                nc.vector.tensor_scalar(
                                    out=sh[:], in0=iota_sb[:],
                                    scalar1=lrow_sb[:, t:t + 1],
                                    scalar2=evals_sb[:, t:t + 1],
                                    op0=mybir.AluOpType.is_equal,
                                    op1=mybir.AluOpType.mult)
                                gt, kg = tile_src[t]
                                nc.tensor.matmul(
                                    out=ps[:], lhsT=sh[:],
                                    rhs=gt[:, kg * DP:kg * DP + D],
                                    start=(j == 0), stop=False)
                            # bias add on PE: identity lhsT x broadcast bias
                            nc.tensor.matmul(
                                out=ps[:], lhsT=ident_sb[:], rhs=biasb_sb[:],
                                start=False, stop=True)
                            # close of the PREVIOUS block: keeps the in-order
                            # DVE queue from stalling on this block's matmuls
                            if pend is not None:
                                close_block(pend[0], pend[1], layer)
                            pend = (b, ps)
                if pend is not None:
                    close_block(pend[0], pend[1], layer)
                    pend = None
                if layer < NL - 1 and not skip_ag:
                    dst = sources[layer + 1]
                    nc.gpsimd.collective_compute(
                        "AllGather", mybir.AluOpType.bypass,
                        replica_groups=rg, ins=[ag_in[:]], outs=[dst[:]])

            # ---- final phase pass 2 ----
            nc.scalar.activation(out=rs_all[:], in_=rr_all[:],
                                 func=mybir.ActivationFunctionType.Sqrt,
                                 scale=sc)
            nc.vector.reciprocal(out=rs_all[:], in_=rs_all[:])
            for b in range(NB):
                rows_b = min(RPC - b * P, P)
                Sb = S_sb[:, b * D:(b + 1) * D]
                F_sb = fpool.tile([P, D], f32, tag="F")
                t2 = fpool.tile([P, 2 * E64], f32, tag="t2")
                nc.vector.tensor_scalar(
                    out=t2[:, 0:E64], in0=Sb[:, E64:2 * E64],
                    scalar1=rs_all[:, 2 * b:2 * b + 1], scalar2=None,
                    op0=mybir.AluOpType.mult)
                nc.vector.tensor_scalar(
                    out=t2[:, E64:2 * E64], in0=Sb[:, 2 * E64:3 * E64],
                    scalar1=rs_all[:, 2 * b + 1:2 * b + 2], scalar2=None,
                    op0=mybir.AluOpType.mult)
                nc.vector.tensor_scalar(
                    out=F_sb[:], in0=Sb[:], scalar1=inv, scalar2=None,
                    op0=mybir.AluOpType.mult)
                nc.vector.tensor_tensor(
                    out=t2[:, 0:E64], in0=t2[:, 0:E64],
                    in1=t2[:, E64:2 * E64], op=mybir.AluOpType.add)
                nc.vector.tensor_tensor(
                    out=F_sb[:, 0:E64], in0=F_sb[:, 0:E64],
                    in1=t2[:, 0:E64], op=mybir.AluOpType.add)
                nc.sync.dma_start(out=Fin[b * P:b * P + rows_b, :],
                                  in_=F_sb[:rows_b, :])

            # ---- owner-computes batch gathers from local Fin ----
            fgt = cpool.tile([P, NHT * D], f32)
            qn = 0
            o = 0
            while o < NH:
                c = min(NH - o, 1024)
                nc.gpsimd.dma_gather(
                    out_ap=fgt[:, o // P * D:(o + c) // P * D].rearrange(
                        "p (t e) -> p t e", t=c // P),
                    in_ap=Fin[:],
                    idxs_ap=bidx_sb[:, o // 16:(o + c) // 16],
                    num_idxs=c,
                    num_idxs_reg=c,
                    elem_size=D,
                    queue_num=qn,
                )
                qn = (qn + 1) % 4
                o += c
            for t in range(NHT):
                nc.sync.dma_start(
                    out=bout[t * P:(t + 1) * P, :],
                    in_=fgt[:, t * D:(t + 1) * D])
    nc.compile()
    return nc


_CACHE = {}


def _get_program(cfg, meta):
    key = (meta["npad"].tobytes(), meta["NH"], cfg.N, cfg.D, cfg.NC)
    if key not in _CACHE:
        _CACHE[key] = build_program(cfg, meta)
    return _CACHE[key]


def run(cfg, inputs):
    meta, in_maps = preprocess(cfg, inputs)
    nc = _get_program(cfg, meta)
    res = run_bass_kernel_spmd(nc, in_maps, core_ids=list(range(cfg.NC)))
    return assemble(cfg, meta, res.results)


def assemble(cfg, meta, results):
    D, E64 = cfg.D, cfg.embed
    B = cfg.batch
    full = np.zeros((3 * B, D), np.float32)
    for c in range(cfg.NC):
        sel = meta["hb"][c]
        full[sel] = results[c]["bout"][:len(sel)]
    out = []
    for part in range(3):          # combined, mean_img, mean_txt
        for s in range(3):         # user, pos, neg
            out.append(np.ascontiguousarray(
                full[s * B:(s + 1) * B, part * E64:(part + 1) * E64]))
    return tuple(out)


def kernel(**inputs):
    cfg = Cfg()
    return run(cfg, inputs)
